# revision 21
# baseline (speedup 1.0000x reference)
"""GCN 2-layer message-passing kernel for Trainium2 (8 NeuronCores, Bass/Tile).

Strategy (graph/data parallel):
  - Nodes partitioned into 8 contiguous ranges (6250 per core, padded 6272).
  - Host does INTEGER/index prep only: bucket edges by (dst core, dst
    block), split by source-row class, sort, build gather-index + dst-slot
    metadata, integer in-degree counts. All FP math runs on device.
  - The halo table g = dinv * (x @ W) is split into TWO source-row CLASSES
    (local rows [0,3072) and [3072,6272)); each class is AllGathered
    separately so the class-0 table is available while the tail of the
    producing phase still runs. This keeps the GpSimd engine (whose
    software descriptor generation for dma_gather is the kernel's hard
    bottleneck at ~8ns/row) busy continuously across layer boundaries.
  - Per layer, per chunk of 7 destination blocks: one dma_gather per class
    fetches all message rows; per 128-edge tile a 0/1 selection matrix
    (DVE is_equal vs iota) scatter-reduces messages into the block's PSUM
    accumulator via one PE matmul; self-loops via an identity matmul;
    bias+residual accumulate in a second PSUM bank; dst-degree scaling via
    ACT per-partition scale. Layer-2 g-table rows are produced inside the
    layer-1 block loop and AllGathered class-by-class as soon as ready.
  - A tiny warm-up dma_gather at t=0 absorbs the Q7 ucode first-call cost.

kernel(**inputs) takes FULL inputs and returns the FULL [50000, 128]
float32 output.
"""
import sys
from contextlib import ExitStack

import numpy as np

if '/opt/trn_rl_repo' not in sys.path:
    sys.path.insert(0, '/opt/trn_rl_repo')

import ml_dtypes

from concourse import bacc, mybir, tile
from concourse.bass_utils import run_bass_kernel_spmd
from concourse.vector_clock import ScopedClock


def _patched_drain_and_barrier(self, tick_clock, wait_clock):
    """Split the kernel-tail drain's sem waits across single-wait drains:
    walrus's NO_STRUCT codegen rejects >1 sync wait on InstDrain."""
    drain_inst = self.nc.sync.drain()
    wait_clock.add_sem_waits(drain_inst.ins,
                             ScopedClock({None: tick_clock.global_clock}))
    si = drain_inst.ins.sync_info
    if si is not None and si.on_wait is not None and len(si.on_wait) > 1:
        waits = list(si.on_wait)
        del si.on_wait[1:]
        for w in waits[1:]:
            d2 = self.nc.sync.drain()
            si2 = d2.ins.sync_info
            if si2 is None:
                d2.ins.sync_info = mybir.SyncInfo(on_wait=[w], on_update=[])
            else:
                si2.on_wait.append(w)
    self.nc.all_engine_barrier()
    assert self.sems is not None
    popped = self.nc._tile_sem_poison_stack.pop()
    assert popped is self._sem_poison
    self.nc.clear_and_free_semaphores(list(self.sems.allocated().values()))
    self.nc.all_engine_barrier()


tile.TileContext._drain_and_barrier = _patched_drain_and_barrier


def split_sync_waits(nc, max_waits=1):
    """Walrus codegen rejects >1 sync wait on several instruction encodings.
    Hoist excess waits onto same-engine no-ops placed just before."""
    import bass_rust
    try:
        funcs = list(nc.m.functions)
    except Exception:
        funcs = [nc.main_func]
    seen = 0
    for fn in funcs:
        for bb in fn.blocks:
            insts = bb.instructions
            new = []
            for ins in insts:
                si = ins.sync_info
                if si is not None and si.on_wait and len(si.on_wait) > max_waits:
                    waits = list(si.on_wait)
                    extra, keep = waits[:-max_waits], waits[-max_waits:]
                    for w in extra:
                        nop = bass_rust.InstNoOp(
                            name=f"I-waitsplit-{seen}", ins=[], outs=[])
                        seen += 1
                        nop.engine = ins.engine
                        nop.sync_info = mybir.SyncInfo(on_wait=[w], on_update=[])
                        new.append(nop)
                    del si.on_wait[:]
                    si.on_wait.extend(keep)
                new.append(ins)
            insts[:] = new
    return seen


bf16 = ml_dtypes.bfloat16
P = 128          # partitions / tile edge
C = 8            # cores
D = 128          # hidden dim
NCLS = 2         # source-row classes
CLS_BLK = (32, 17)           # blocks per class (32*128=4096, 17*128=2176)
CLS_BASE = (0, 4096)
CLS_SZ = (4096, 2176)        # class-0 table = 8*4096 = 32768 rows (int16 max)
CB = 5           # dst blocks per gather chunk


# ---------------------------------------------------------------------------
# Host-side integer/index prep (sharding + metadata; no FP math on values)
# ---------------------------------------------------------------------------

def prep(edge_index, n_nodes):
    N = n_nodes
    npc = N // C
    assert npc * C == N
    B = (npc + P - 1) // P
    npad = B * P
    assert B == CLS_BLK[0] + CLS_BLK[1] and npad == CLS_SZ[0] + CLS_SZ[1]

    ei = np.asarray(edge_index)
    src_all = ei[0].astype(np.int64)
    dst_all = ei[1].astype(np.int64)
    # self-loops handled on-device via identity matmul; count in degree
    deg_all = np.bincount(dst_all, minlength=N) + 1

    own_s = src_all // npc
    loc_s = src_all - own_s * npc
    cls_all = (loc_s >= CLS_SZ[0]).astype(np.int64)
    row_all = np.where(cls_all == 0,
                       own_s * CLS_SZ[0] + loc_s,
                       own_s * CLS_SZ[1] + (loc_s - CLS_BASE[1]))

    owner_all = dst_all // npc
    per_core = []
    cnt = np.zeros((C, NCLS, B), dtype=np.int64)
    for c in range(C):
        m = owner_all == c
        r = row_all[m]
        k = cls_all[m]
        dloc = dst_all[m] - c * npc
        blk = dloc >> 7
        slot = dloc & 127
        order = np.lexsort((r, blk, k))
        r, k, blk, slot = r[order], k[order], blk[order], slot[order]
        per_core.append((r, k, blk, slot))
        for kk in range(NCLS):
            mk = k == kk
            cnt[c, kk] = np.bincount(blk[mk], minlength=B)

    # uniform tile counts: max over cores, per (class, block)
    T = [np.ceil(cnt[:, kk, :].max(axis=0) / P).astype(np.int64)
         for kk in range(NCLS)]
    tile_base = [np.concatenate([[0], np.cumsum(T[kk])]) for kk in range(NCLS)]
    T_total = [int(T[kk].sum()) for kk in range(NCLS)]

    idx = [np.zeros((C, T_total[kk] * P), dtype=np.int64) for kk in range(NCLS)]
    slots = [np.full((C, T_total[kk] * P), -1.0, dtype=np.float32)
             for kk in range(NCLS)]
    for c in range(C):
        r, k, blk, slot = per_core[c]
        for kk in range(NCLS):
            mk = k == kk
            rk, bk, sk = r[mk], blk[mk], slot[mk]
            bstart = np.concatenate([[0], np.cumsum(np.bincount(bk, minlength=B))])
            for b in range(B):
                e0, e1 = bstart[b], bstart[b + 1]
                o = tile_base[kk][b] * P
                idx[kk][c, o:o + (e1 - e0)] = rk[e0:e1]
                slots[kk][c, o:o + (e1 - e0)] = sk[e0:e1]

    deg = np.ones((C, P, B), dtype=np.float32)
    for c in range(C):
        dpad = np.ones(npad, dtype=np.float32)
        dpad[:npc] = deg_all[c * npc:(c + 1) * npc].astype(np.float32)
        deg[c] = dpad.reshape(B, P).T

    def pack16(a):
        # wrapped layout: element j -> [j % 16, j // 16], replicated to the
        # 8 Q7 cores' partition groups (128 partitions total)
        n = a.shape[1]
        w = a.reshape(a.shape[0], n // 16, 16).transpose(0, 2, 1).astype(np.int16)
        return np.tile(w, (1, 8, 1)).copy()

    chunks = []
    for b0 in range(0, B, CB):
        b1 = min(b0 + CB, B)
        chunks.append(dict(
            b0=b0, b1=b1,
            t0=[int(tile_base[kk][b0]) for kk in range(NCLS)],
            t1=[int(tile_base[kk][b1]) for kk in range(NCLS)],
        ))

    return dict(
        npc=npc, npad=npad, B=B,
        T=[T[kk].tolist() for kk in range(NCLS)],
        tile_base=[tile_base[kk].tolist() for kk in range(NCLS)],
        T_total=T_total, chunks=chunks,
        idx=[pack16(idx[kk]) for kk in range(NCLS)],
        slots=[slots[kk].reshape(C, T_total[kk], P).transpose(0, 2, 1).copy()
               for kk in range(NCLS)],
        deg=deg,
    )


# ---------------------------------------------------------------------------
# Device program (uniform across the 8 cores)
# ---------------------------------------------------------------------------

def build_program(meta):
    npad, B = meta['npad'], meta['B']
    T, tile_base, T_total = meta['T'], meta['tile_base'], meta['T_total']
    chunks = meta['chunks']
    f32 = mybir.dt.float32
    bf = mybir.dt.bfloat16
    max_ct = [max(ch['t1'][kk] - ch['t0'][kk] for ch in chunks)
              for kk in range(NCLS)]
    selmax = [max(T[kk]) for kk in range(NCLS)]

    nc = bacc.Bacc(None, target_bir_lowering=False)
    xT_p = nc.declare_dram_parameter("xT", [P, npad], f32, isOutput=False)
    xTf_p = nc.declare_dram_parameter("xTf", [P, C * npad], f32, isOutput=False)
    degf_p = nc.declare_dram_parameter("degf", [P, C * B], f32, isOutput=False)
    w1_p = nc.declare_dram_parameter("w1", [P, D], f32, isOutput=False)
    w2_p = nc.declare_dram_parameter("w2", [P, D], f32, isOutput=False)
    rw_p = nc.declare_dram_parameter("resw", [P, D], f32, isOutput=False)
    cb_p = nc.declare_dram_parameter("convb", [2, D], f32, isOutput=False)
    rb_p = nc.declare_dram_parameter("resb", [1, D], f32, isOutput=False)
    deg_p = nc.declare_dram_parameter("deg", [P, B], f32, isOutput=False)
    idx_p = [nc.declare_dram_parameter(f"idx{kk}", [128, T_total[kk] * 8],
                                       mybir.dt.int16, isOutput=False)
             for kk in range(NCLS)]
    slot_p = [nc.declare_dram_parameter(f"slot{kk}", [P, T_total[kk]], bf,
                                        isOutput=False)
              for kk in range(NCLS)]
    warm_p = nc.declare_dram_parameter("warmidx", [128, 8], mybir.dt.int16,
                                       isOutput=False)
    iota_p = nc.declare_dram_parameter("iota", [P, P], bf, isOutput=False)
    ident_p = nc.declare_dram_parameter("ident", [P, P], f32, isOutput=False)
    ones_p = nc.declare_dram_parameter("ones", [1, D], bf, isOutput=False)
    out_p = nc.declare_dram_parameter("out", [npad, D], f32, isOutput=True)

    g_own = [[nc.dram_tensor(f"g{l}o{kk}", [CLS_SZ[kk], D], bf)
              for kk in range(NCLS)] for l in range(2)]
    # layer-1 tables are computed locally from the replicated x (no
    # collective); layer-2 tables still come from class-split AllGathers
    g_full = [[nc.dram_tensor(f"g0f{kk}", [C * CLS_SZ[kk], D], bf)
               for kk in range(NCLS)],
              [nc.dram_tensor(f"g1f{kk}", [C * CLS_SZ[kk], D], bf,
                              addr_space="Shared")
               for kk in range(NCLS)]]

    with tile.TileContext(nc) as tc, ExitStack() as ctx:
        const = ctx.enter_context(tc.tile_pool(name="const", bufs=1))
        gbuf = ctx.enter_context(tc.tile_pool(name="gbuf", bufs=3))
        work = ctx.enter_context(tc.tile_pool(name="work", bufs=6))
        outp = ctx.enter_context(tc.tile_pool(name="outp", bufs=3))
        psum = ctx.enter_context(tc.tile_pool(name="psum", bufs=2, space="PSUM"))

        # ---- warm-up gather: absorb Q7 ucode first-call cost immediately
        warmidx = const.tile([128, 8], mybir.dt.int16)
        nc.sync.dma_start(out=warmidx[:], in_=warm_p[:, :])
        warmg = const.tile([P, 1, D], bf)
        nc.gpsimd.dma_gather(out_ap=warmg[:], in_ap=g_full[0][0][:, :],
                             idxs_ap=warmidx[:], num_idxs=128,
                             num_idxs_reg=128, elem_size=D,
                             single_packet=False)

        # ---- constants / persistent state ----
        xT = const.tile([P, npad], f32)
        nc.sync.dma_start(out=xT[:], in_=xT_p[:, :])
        x1T = const.tile([P, npad], bf)          # layer-1 output, transposed
        w1 = const.tile([P, D], f32)
        nc.sync.dma_start(out=w1[:], in_=w1_p[:, :])
        w2f = const.tile([P, D], f32)
        nc.sync.dma_start(out=w2f[:], in_=w2_p[:, :])
        rwf = const.tile([P, D], f32)
        nc.sync.dma_start(out=rwf[:], in_=rw_p[:, :])
        w2b = const.tile([P, D], bf)
        nc.vector.tensor_copy(out=w2b[:], in_=w2f[:])
        rwb = const.tile([P, D], bf)
        nc.vector.tensor_copy(out=rwb[:], in_=rwf[:])

        rb = const.tile([1, D], f32)
        nc.sync.dma_start(out=rb[:], in_=rb_p[:, :])
        bcomb = []
        for l in range(2):
            cbl = const.tile([1, D], f32, tag=f"cb{l}")
            nc.sync.dma_start(out=cbl[:], in_=cb_p[l:l + 1, :])
            bc = const.tile([1, D], bf, tag=f"bcomb{l}")
            nc.vector.tensor_tensor(out=bc[:], in0=cbl[:], in1=rb[:],
                                    op=mybir.AluOpType.add)
            bcomb.append(bc)
        ones1 = const.tile([1, D], bf)
        nc.sync.dma_start(out=ones1[:], in_=ones_p[:, :])

        iota = const.tile([P, P], bf)
        nc.sync.dma_start(out=iota[:], in_=iota_p[:, :])
        ident = const.tile([P, P], f32)
        nc.sync.dma_start(out=ident[:], in_=ident_p[:, :])
        ident_bf = const.tile([P, P], bf)
        nc.vector.tensor_copy(out=ident_bf[:], in_=ident[:])

        idxt = []
        slots = []
        for kk in range(NCLS):
            it = const.tile([128, T_total[kk] * 8], mybir.dt.int16, tag=f"idx{kk}")
            nc.sync.dma_start(out=it[:], in_=idx_p[kk][:, :])
            idxt.append(it)
            st = const.tile([P, T_total[kk]], bf, tag=f"slot{kk}")
            nc.sync.dma_start(out=st[:], in_=slot_p[kk][:, :])
            slots.append(st)

        degt = const.tile([P, B], f32)
        nc.sync.dma_start(out=degt[:], in_=deg_p[:, :])
        sdeg = const.tile([P, B], f32)
        nc.scalar.activation(out=sdeg[:], in_=degt[:],
                             func=mybir.ActivationFunctionType.Sqrt)
        dinv = const.tile([P, B], f32)
        nc.vector.reciprocal(out=dinv[:], in_=sdeg[:])
        degft = const.tile([P, C * B], f32)
        nc.sync.dma_start(out=degft[:], in_=degf_p[:, :])
        sdegf = const.tile([P, C * B], f32)
        nc.scalar.activation(out=sdegf[:], in_=degft[:],
                             func=mybir.ActivationFunctionType.Sqrt)
        dinvf = const.tile([P, C * B], f32)
        nc.vector.reciprocal(out=dinvf[:], in_=sdegf[:])

        # ---- phase 1: build the FULL layer-1 g table locally (x is
        # replicated to every core, so no collective is needed at all).
        # Class-1 (17 blocks x 8 cores) is built first so the layer-1
        # class-1 gathers can start ~30us in; class-0 follows while those
        # gathers run.
        xs = ctx.enter_context(tc.tile_pool(name="xs", bufs=3))
        with nc.named_scope("phase1"):
            for kk in (1, 0):
                nblk_ch = (16, 17)[kk]           # blocks per stream chunk
                nch = (2, 1)[kk]                 # chunks per core
                for o in range(C):
                    for q in range(nch):
                        blk0 = q * nblk_ch
                        nb = min(nblk_ch, CLS_BLK[kk] - blk0)
                        col0 = o * npad + CLS_BASE[kk] + blk0 * P
                        xt = xs.tile([P, 17 * P], f32, tag="xs")
                        nc.sync.dma_start(out=xt[:, :nb * P],
                                          in_=xTf_p[:, col0:col0 + nb * P])
                        stg = xs.tile([P, 17, D], bf, tag="pstg")
                        for j in range(nb):
                            gcol = o * B + CLS_BASE[kk] // P + blk0 + j
                            ph = psum.tile([P, D], f32, tag="ph")
                            nc.tensor.matmul(out=ph[:],
                                             lhsT=xt[:, j * P:(j + 1) * P],
                                             rhs=w1[:], start=True, stop=True)
                            nc.scalar.activation(
                                out=stg[:, j, :], in_=ph[:],
                                func=mybir.ActivationFunctionType.Copy,
                                scale=dinvf[:, gcol:gcol + 1])
                        rb = (o * CLS_SZ[kk]) // P + blk0
                        nc.sync.dma_start(
                            out=g_full[0][kk]
                                .reshape([C * CLS_SZ[kk] // P, P, D])
                                .transpose([1, 0, 2])[:, rb:rb + nb, :],
                            in_=stg[:, :nb, :])

        def emit_gather(l, kk, ch):
            nt = ch['t1'][kk] - ch['t0'][kk]
            if nt == 0:
                return None
            gt = gbuf.tile([P, max_ct[kk], D], bf, tag=f"g{kk}")
            nc.gpsimd.dma_gather(
                out_ap=gt[:, :nt, :], in_ap=g_full[l][kk][:, :],
                idxs_ap=idxt[kk][:, ch['t0'][kk] * 8:ch['t1'][kk] * 8],
                num_idxs=nt * P, num_idxs_reg=nt * P, elem_size=D,
                single_packet=False)
            return gt

        def emit_block(l, b, ch, gts):
            cs = slice(b * P, (b + 1) * P)
            pB = psum.tile([P, D], f32, tag="pB")
            nc.tensor.matmul(out=pB[:], lhsT=ones1[:], rhs=bcomb[l][:],
                             start=True, stop=False)
            if l == 0:
                nc.tensor.matmul(out=pB[:], lhsT=xT[:, cs], rhs=rwf[:],
                                 start=False, stop=True)
            else:
                nc.tensor.matmul(out=pB[:], lhsT=x1T[:, cs], rhs=rwb[:],
                                 start=False, stop=True)
            pA = psum.tile([P, D], f32, tag="pA")
            # self-loop: psum += I @ (own g rows)
            kb = 0 if b < CLS_BLK[0] else 1
            lo = b * P - CLS_BASE[kb]
            gsb = work.tile([P, D], bf, tag="gsb")
            if l == 0:
                # recompute locally (g1 own rows are only in the shared
                # table at a core-dependent offset)
                phg = psum.tile([P, D], f32, tag="ph")
                nc.tensor.matmul(out=phg[:], lhsT=xT[:, cs], rhs=w1[:],
                                 start=True, stop=True)
                nc.scalar.activation(out=gsb[:], in_=phg[:],
                                     func=mybir.ActivationFunctionType.Copy,
                                     scale=dinv[:, b:b + 1])
            else:
                nc.sync.dma_start(out=gsb[:], in_=g_own[l][kb][lo:lo + P, :])
            ntot = T[0][b] + T[1][b]
            nc.tensor.matmul(out=pA[:], lhsT=ident_bf[:], rhs=gsb[:],
                             start=True, stop=(ntot == 0))
            done = 0
            for kk in range(NCLS):
                ntk = T[kk][b]
                if ntk == 0:
                    continue
                tb = tile_base[kk][b]
                toff = tb - ch['t0'][kk]
                selb = work.tile([P, selmax[kk], P], bf, tag=f"sel{kk}")
                nc.vector.tensor_tensor(
                    out=selb[:, :ntk, :],
                    in0=slots[kk][:, tb:tb + ntk]
                        .rearrange("p (k o) -> p k o", o=1)
                        .to_broadcast([P, ntk, P]),
                    in1=iota[:].rearrange("p (o d) -> p o d", o=1)
                        .to_broadcast([P, ntk, P]),
                    op=mybir.AluOpType.is_equal)
                for t in range(ntk):
                    done += 1
                    nc.tensor.matmul(out=pA[:], lhsT=selb[:, t, :],
                                     rhs=gts[kk][:, toff + t, :],
                                     start=False, stop=(done == ntot))
            t1 = outp.tile([P, D], f32, tag="t1")
            nc.scalar.activation(out=t1[:], in_=pA[:],
                                 func=mybir.ActivationFunctionType.Copy,
                                 scale=dinv[:, b:b + 1])
            t2 = outp.tile([P, D], f32, tag="t2")
            nc.vector.tensor_tensor(out=t2[:], in0=t1[:], in1=pB[:],
                                    op=mybir.AluOpType.add)
            xo = outp.tile([P, D], f32, tag="xo")
            nc.scalar.activation(out=xo[:], in_=t2[:],
                                 func=mybir.ActivationFunctionType.Relu)
            if l == 0:
                pT = psum.tile([P, D], f32, tag="pT")
                nc.tensor.transpose(out=pT[:], in_=xo[:], identity=ident[:])
                nc.vector.tensor_copy(out=x1T[:, cs], in_=pT[:])
                ph2 = psum.tile([P, D], f32, tag="ph")
                nc.tensor.matmul(out=ph2[:], lhsT=x1T[:, cs], rhs=w2b[:],
                                 start=True, stop=True)
                g2b = outp.tile([P, D], bf, tag="gb")
                nc.scalar.activation(out=g2b[:], in_=ph2[:],
                                     func=mybir.ActivationFunctionType.Copy,
                                     scale=dinv[:, b:b + 1])
                nc.sync.dma_start(out=g_own[1][kb][lo:lo + P, :], in_=g2b[:])
            else:
                nc.sync.dma_start(out=out_p[cs, :], in_=xo[:])

        # ---- layer 1 ----
        with nc.named_scope("layer1"):
            for ci, ch in enumerate(chunks):
                gt1 = emit_gather(0, 1, ch)
                gts = [emit_gather(0, 0, ch), gt1]
                if ci == 8:
                    # class-0 g2 rows (blocks 0..31) are written by now;
                    # the collective runs while the last chunks' gathers go
                    with nc.named_scope("ag2a"):
                        nc.gpsimd.collective_compute(
                            "AllGather", mybir.AluOpType.bypass,
                            replica_groups=[list(range(C))],
                            ins=[g_own[1][0][:, :]], outs=[g_full[1][0][:, :]])
                for b in range(ch['b0'], ch['b1']):
                    emit_block(0, b, ch, gts)
        # ---- layer 2 (class-1 AllGather slotted after the first gather) ----
        with nc.named_scope("layer2"):
            for ci, ch in enumerate(chunks):
                gts = [emit_gather(1, 0, ch)]
                if ci == 0:
                    with nc.named_scope("ag2b"):
                        nc.gpsimd.collective_compute(
                            "AllGather", mybir.AluOpType.bypass,
                            replica_groups=[list(range(C))],
                            ins=[g_own[1][1][:, :]], outs=[g_full[1][1][:, :]])
                gts.append(emit_gather(1, 1, ch))
                for b in range(ch['b0'], ch['b1']):
                    emit_block(1, b, ch, gts)
    return nc


# ---------------------------------------------------------------------------
# Entry point
# ---------------------------------------------------------------------------

def make_inputs(x, conv_w, conv_b, res_w, res_b, meta):
    npc, npad = meta['npc'], meta['npad']
    iota = np.tile(np.arange(P, dtype=np.float32), (P, 1)).astype(bf16)
    warm = np.zeros((128, 8), dtype=np.int16)
    xTf_full = np.zeros((P, C * npad), dtype=np.float32)
    xf = np.asarray(x, dtype=np.float32)
    for o in range(C):
        xTf_full[:, o * npad:o * npad + npc] = xf[o * npc:(o + 1) * npc].T
    degf_full = np.concatenate([meta['deg'][o] for o in range(C)], axis=1)
    in_maps = []
    for c in range(C):
        xT = np.zeros((P, npad), dtype=np.float32)
        xT[:, :npc] = np.asarray(x[c * npc:(c + 1) * npc], dtype=np.float32).T
        in_maps.append({
            "xT": xT,
            "xTf": xTf_full,
            "degf": degf_full,
            "w1": np.asarray(conv_w[0], dtype=np.float32),
            "w2": np.asarray(conv_w[1], dtype=np.float32),
            "resw": np.asarray(res_w, dtype=np.float32),
            "convb": np.asarray(conv_b, dtype=np.float32),
            "resb": np.asarray(res_b, dtype=np.float32).reshape(1, D),
            "deg": meta['deg'][c],
            "idx0": meta['idx'][0][c],
            "idx1": meta['idx'][1][c],
            "slot0": meta['slots'][0][c].astype(bf16),
            "slot1": meta['slots'][1][c].astype(bf16),
            "warmidx": warm,
            "iota": iota,
            "ident": np.eye(P, dtype=np.float32),
            "ones": np.ones((1, D), dtype=np.float32).astype(bf16),
        })
    return in_maps


def run(x, edge_index, conv_w, conv_b, res_w, res_b, trace=False, trace_kwargs=None):
    N = x.shape[0]
    meta = prep(edge_index, N)
    nc = build_program(meta)
    nc.compile()
    split_sync_waits(nc)
    in_maps = make_inputs(x, conv_w, conv_b, res_w, res_b, meta)
    res = run_bass_kernel_spmd(nc, in_maps, list(range(C)), trace=trace,
                               **(trace_kwargs or {}))
    npc = meta['npc']
    out = np.concatenate([np.asarray(res.results[c]["out"])[:npc]
                          for c in range(C)], axis=0)
    return out.astype(np.float32), res


def kernel(x, edge_index, conv_w, conv_b, res_w, res_b):
    out, _ = run(x, edge_index, conv_w, conv_b, res_w, res_b, trace=False)
    return out
